# revision 21
# baseline (speedup 1.0000x reference)
"""MoE layer (hash-routed, top-k=2, E=8 experts) on 8 Trainium2 NeuronCores.

Strategy: expert-parallel. Core e holds expert e's weights (W1[e], W2[e]).
The host routes: for each expert, gather the distinct tokens assigned to it
(assign[b,s,:] contains expert ids; a token contributes once per distinct
expert), transpose the gathered activations to [D, C] so the device never
has to transpose, run a dense 2-layer MLP per core, then scatter-add the
per-expert outputs back and divide by k.

Device kernel (per core), C = token capacity (padded, multiple of 128):
  for each quarter q of H (HQ columns):
    layer1: H1T[h, tok] = relu(W1q^T @ XT + b1q)   (PSUM-accum over d-tiles)
    layer2: Y[tok, d]  += H1T^T @ W2q              (PSUM-accum over h-tiles,
                                                    SBUF f32 accum across q)
All matmul operands are bf16 (accumulation stays f32 in PSUM): walrus then
emits separate LDWEIGHTS instructions that the PE pulls into the background
weight buffer, so the ~128-cycle weight load hides under the previous
matmul's streaming instead of serializing with it (f32r matmuls are
self-loading and pay that cost on every MM). W1 is host-retiled to
[ht, p, kt, h] so each h-tile's weights load as one DMA with 2KB
contiguous lines; b1 is host-swizzled so the bias tile loads as 128
contiguous lines instead of 4096 4-byte descriptors.
"""

import math
import numpy as np
import ml_dtypes

import concourse.bass as bass
import concourse.mybir as mybir
import concourse.tile as tile
from concourse import bacc
from concourse.bass_utils import run_bass_kernel_spmd

dt = mybir.dt

B, S, D, H, E, NCORES = 4, 1024, 1024, 4096, 8, 8
HQ = 1024                      # h-quarter width
KT = D // 128                  # 8 contraction tiles (d)
HTQ = HQ // 128                # 8 h-tiles per quarter
NQ = H // HQ                   # 4 quarters
# PE warm-up matmuls: enough continuous PE-busy time (~3.4us) to flip the
# HAM clock gate to 8/8 BEFORE the first data-dependent matmul, and enough
# to bridge the DMA prologue without the PE idling in between — an idle
# gap resets the HAM busy window and leaves the whole first quarter
# running at 1.2 GHz
WARMUP_MMS = 7

BF16 = np.dtype(ml_dtypes.bfloat16)

_BUILD_CACHE: dict = {}


def build_nc(C: int):
    """Build + compile the per-core Bass program for token capacity C."""
    assert C % 128 == 0
    assert NQ >= 2  # final-quarter y store lives in the add branch
    TT = C // 128

    nc = bacc.Bacc(
        "TRN2",
        target_bir_lowering=False,
        debug=False,
        num_devices=NCORES,
    )

    xt_d = nc.dram_tensor("xt", [D, C], dt.bfloat16, kind="ExternalInput")
    # host-retiled: w1[ht, p, kt, h] = W1[kt*128+p, ht*128+h]
    w1_d = nc.dram_tensor(
        "w1", [H // 128, 128, KT, 128], dt.bfloat16, kind="ExternalInput"
    )
    # host-swizzled: b1s[p*32+ht] = b1[ht*128+p]
    b1_d = nc.dram_tensor("b1", [H], dt.float32, kind="ExternalInput")
    w2_d = nc.dram_tensor("w2", [H, D], dt.bfloat16, kind="ExternalInput")
    y_d = nc.dram_tensor("y", [C, D], dt.float32, kind="ExternalOutput")

    xt_v = xt_d.ap().rearrange("(kt p) c -> p kt c", p=128)
    b1_v = b1_d.ap().rearrange("(p ht) -> p ht", p=128)
    y_v = y_d.ap().rearrange("(tt p) d -> p tt d", p=128)
    w2_v = w2_d.ap().rearrange("(hh p) d -> p hh d", p=128)
    w1_4v = w1_d.ap().rearrange("a p k h -> p a k h")

    # SBUF per-partition budget check (bytes)
    need = (
        KT * C * 2                # xt (bf16)
        + TT * 1024 * 4           # y (f32)
        + 2 * HTQ * KT * 128 * 2  # w1 quarters (double-buffered)
        + 2 * HTQ * 1024 * 2      # w2 quarters (double-buffered)
        + 2 * HTQ * C * 2         # h1q (double-buffered)
        + 32 * 4                  # b1
        + 512 * 2                 # warm tile
    )
    assert need <= 200 * 1024, f"SBUF over budget: {need // 1024}KB for C={C}"

    # xt arrives as fused all-kt column-chunk DMAs; the leading chunks are
    # fine so the first L1 h-tile can start computing before the bulk lands
    xt_chunks = [(0, 128), (128, 128), (256, 128), (384, 128)]
    c0 = 512
    while c0 < C:
        n = min(512, C - c0)
        xt_chunks.append((c0, n))
        c0 += n
    n_chunks = [(c0, min(512, C - c0)) for c0 in range(0, C, 512)]

    with tile.TileContext(nc) as tc:
        with (
            tc.tile_pool(name="xt", bufs=1) as xt_pool,
            tc.tile_pool(name="b1", bufs=1) as b1_pool,
            tc.tile_pool(name="y", bufs=1) as y_pool,
            tc.tile_pool(name="w1q", bufs=2) as w1_pool,
            tc.tile_pool(name="w2q", bufs=2) as w2_pool,
            tc.tile_pool(name="h1q", bufs=2) as h1_pool,
            tc.tile_pool(name="ps1", bufs=4, space="PSUM") as ps1_pool,
            tc.tile_pool(name="ps2", bufs=4, space="PSUM") as ps2_pool,
        ):
            # PE warm-up: dependency-free bf16 matmuls issued during the
            # initial DMA prologue so the HAM clock gate reaches 8/8
            # (2.4 GHz) before the first real matmul. The warm tile borrows
            # the y pool's buffer (y is first written long after the warm-up
            # matmuls retire) and the warm PSUM tile rotates through ps2 —
            # no dedicated pools, which keeps the scheduler's end-of-program
            # semaphore quiesce short.
            wt = y_pool.tile([128, 512], dt.bfloat16)
            nc.vector.memset(wt[:], 0.0)
            wps = ps2_pool.tile([128, 512], dt.float32, tag="ps2")
            for _ in range(WARMUP_MMS):
                nc.tensor.matmul(wps[:], wt[:, :128], wt[:], start=True, stop=True)

            # xt chunks are interleaved with the w1 quarter-0 loads below in
            # exact consumption order, all on the sync (HWDGE) ring; the
            # gpsimd SWDGE path stays completely unused. b1 (16KB) rides
            # along early (first activation needs it ~1us after the first
            # real matmul group).
            b1t = b1_pool.tile([128, H // 128], dt.float32)
            xt = xt_pool.tile([128, KT, C], dt.bfloat16)
            y = y_pool.tile([128, TT, 1024], dt.float32)

            # q=0 layer 1 runs chunk-major: sweep columns 0-512 over every
            # h-tile first (fine chunks on ht=0 so compute starts as soon as
            # the first 128 columns land), then the 512+ columns — by then
            # the xt bulk chunks have long finished streaming
            fine_chunks = [(c0, n) for c0, n in xt_chunks if c0 < 512]

            def l1_group(w1q, h1q, q, ht, c0, n):
                hidx = q * HTQ + ht
                ps = ps1_pool.tile([128, 512], dt.float32, tag="ps1")
                for kt in range(KT):
                    nc.tensor.matmul(
                        ps[:, :n],
                        w1q[:, ht, kt, :],
                        xt[:, kt, c0 : c0 + n],
                        start=(kt == 0),
                        stop=(kt == KT - 1),
                    )
                nc.scalar.activation(
                    h1q[:, ht, c0 : c0 + n],
                    ps[:, :n],
                    mybir.ActivationFunctionType.Relu,
                    bias=b1t[:, hidx : hidx + 1],
                )

            for q in range(NQ):
                w2q = w2_pool.tile([128, HTQ, 1024], dt.bfloat16)
                h1q = h1_pool.tile([128, HTQ, C], dt.bfloat16)

                # ---- layer 1: H1T[h, tok] = relu(W1q^T @ XT + b1) ----
                w1q = w1_pool.tile([128, HTQ, KT, 128], dt.bfloat16)
                if q == 0:
                    # prologue rides BOTH HWDGE rings (sync + scalar) in
                    # parallel, each in pass-A consumption order — the
                    # serialized early transfers are the ramp critical path
                    fine = [c for c in xt_chunks if c[0] < 512]
                    bulk = [c for c in xt_chunks if c[0] >= 512]

                    def _xt(c):
                        c0, n = c
                        return (xt[:, :, c0 : c0 + n], xt_v[:, :, c0 : c0 + n])

                    sync_ops = [
                        _xt(fine[0]),
                        (w1q[:, 0], w1_d.ap()[0]),
                        _xt(fine[2]),
                        (w1q[:, 2], w1_d.ap()[2]),
                        (w1q[:, 4], w1_d.ap()[4]),
                        _xt(bulk[0]),
                        (w1q[:, 6], w1_d.ap()[6]),
                    ]
                    scalar_ops = [
                        (b1t[:], b1_v),
                        _xt(fine[1]),
                        (w1q[:, 1], w1_d.ap()[1]),
                        _xt(fine[3]),
                        (w1q[:, 3], w1_d.ap()[3]),
                        (w1q[:, 5], w1_d.ap()[5]),
                        (w1q[:, 7], w1_d.ap()[7]),
                    ] + [_xt(c) for c in bulk[1:]]
                    for i in range(max(len(sync_ops), len(scalar_ops))):
                        if i < len(sync_ops):
                            nc.sync.dma_start(*sync_ops[i])
                        if i < len(scalar_ops):
                            nc.scalar.dma_start(*scalar_ops[i])
                else:
                    # later quarters load 4 h-tiles per DMA — nothing is
                    # latency-critical there, and fewer ops cost less
                    # sequencer time
                    for ht in range(0, HTQ, 4):
                        nc.sync.dma_start(
                            w1q[:, ht : ht + 4], w1_4v[:, q * HTQ + ht : q * HTQ + ht + 4]
                        )
                if q == 0:
                    for ht in range(HTQ):
                        for c0, n in fine_chunks if ht == 0 else n_chunks[:1]:
                            l1_group(w1q, h1q, q, ht, c0, n)
                    for ht in range(HTQ):
                        for c0, n in n_chunks[1:]:
                            l1_group(w1q, h1q, q, ht, c0, n)
                else:
                    for ht in range(HTQ):
                        for c0, n in n_chunks:
                            l1_group(w1q, h1q, q, ht, c0, n)

                # w2 quarter loads (4 h-tiles per DMA), emitted after layer
                # 1 so they never outprioritize the w1 stream on the sync
                # queue; they land well before layer 2 needs them
                for ht in range(0, HTQ, 4):
                    nc.sync.dma_start(
                        w2q[:, ht : ht + 4, :],
                        w2_v[:, q * HTQ + ht : q * HTQ + ht + 4, :],
                    )

                # ---- layer 2: Y[tok, d] += H1T^T @ W2q ----
                for tt in range(TT):
                    for dc in range(2):
                        ps = ps2_pool.tile([128, 512], dt.float32, tag="ps2")
                        for ht in range(HTQ):
                            nc.tensor.matmul(
                                ps[:],
                                h1q[:, ht, tt * 128 : (tt + 1) * 128],
                                w2q[:, ht, dc * 512 : (dc + 1) * 512],
                                start=(ht == 0),
                                stop=(ht == HTQ - 1),
                            )
                        ys = y[:, tt, dc * 512 : (dc + 1) * 512]
                        if q == 0:
                            nc.vector.tensor_copy(ys, ps[:])
                        else:
                            nc.vector.tensor_add(ys, ys, ps[:])
                            if q == NQ - 1:
                                # stores ride the scalar (ACT) HWDGE ring —
                                # a separate physical ring from the sync
                                # one, so they never queue ahead of the
                                # w1/w2 weight stream
                                nc.scalar.dma_start(
                                    y_v[:, tt, dc * 512 : (dc + 1) * 512], ys
                                )

    nc.compile()
    return nc


def _get_nc(C: int):
    if C not in _BUILD_CACHE:
        _BUILD_CACHE[C] = build_nc(C)
    return _BUILD_CACHE[C]


def _retile_w1(w1e: np.ndarray) -> np.ndarray:
    # [D, H] -> [ht, p, kt, h] with w1[ht, p, kt, h] = W1[kt*128+p, ht*128+h]
    return np.ascontiguousarray(
        w1e.reshape(KT, 128, H // 128, 128).transpose(2, 1, 0, 3).astype(BF16)
    )


def kernel(x, W1, b1, W2, b2, assign, k, _want_trace=False):
    x = np.asarray(x, dtype=np.float32)
    W1 = np.asarray(W1, dtype=np.float32)
    b1 = np.asarray(b1, dtype=np.float32)
    W2 = np.asarray(W2, dtype=np.float32)
    b2 = np.asarray(b2, dtype=np.float32)
    assign = np.asarray(assign)
    kk = int(k)

    assert W1.shape[0] == E and W2.shape[0] == E, "expert count must be 8"
    Bx, Sx, Dx = x.shape
    T = Bx * Sx
    xf = x.reshape(T, Dx)
    xT = np.ascontiguousarray(xf.T.astype(BF16))  # [D, T] bf16
    a2 = assign.reshape(T, -1)

    idx = [np.nonzero((a2 == e).any(axis=1))[0] for e in range(E)]
    max_n = max(len(i) for i in idx)

    # capacity per device pass (multiple of 128); single pass for the
    # expected distribution, multiple passes if pathologically skewed
    C = min(max(1024, math.ceil(max_n / 128) * 128), 1280)
    n_pass = math.ceil(max(max_n, 1) / C)

    nc = _get_nc(C)

    w1_io = [_retile_w1(W1[e]) for e in range(E)]
    w2_io = [np.ascontiguousarray(W2[e].astype(BF16)) for e in range(E)]
    # b1s[p*32+ht] = b1[ht*128+p]
    b1_io = [
        np.ascontiguousarray(b1[e].reshape(H // 128, 128).T).reshape(H)
        for e in range(E)
    ]

    out_f = np.zeros((T, Dx), dtype=np.float32)
    trace_info = None

    for p in range(n_pass):
        in_maps = []
        for e in range(E):
            sl = idx[e][p * C : (p + 1) * C]
            xt_buf = np.zeros((Dx, C), dtype=BF16)
            if len(sl):
                xt_buf[:, : len(sl)] = xT[:, sl]
            in_maps.append(
                {
                    "xt": xt_buf,
                    "w1": w1_io[e],
                    "b1": b1_io[e],
                    "w2": w2_io[e],
                }
            )
        res = run_bass_kernel_spmd(
            nc,
            in_maps,
            core_ids=list(range(NCORES)),
            trace=_want_trace,
            trace_cores=list(range(NCORES)) if _want_trace else None,
        )
        if _want_trace:
            trace_info = res
        for e in range(E):
            sl = idx[e][p * C : (p + 1) * C]
            if len(sl):
                out_f[sl] += res.results[e]["y"][: len(sl)] + b2[e][None, :]

    out = (out_f * np.float32(1.0 / kk)).reshape(Bx, Sx, Dx)
    if _want_trace:
        return out, trace_info
    return out


# revision 22
# speedup vs baseline: 1.0101x; 1.0101x over previous
"""MoE layer (hash-routed, top-k=2, E=8 experts) on 8 Trainium2 NeuronCores.

Strategy: expert-parallel. Core e holds expert e's weights (W1[e], W2[e]).
The host routes: for each expert, gather the distinct tokens assigned to it
(assign[b,s,:] contains expert ids; a token contributes once per distinct
expert), transpose the gathered activations to [D, C] so the device never
has to transpose, run a dense 2-layer MLP per core, then scatter-add the
per-expert outputs back and divide by k.

Device kernel (per core), C = token capacity (padded, multiple of 128):
  for each quarter q of H (HQ columns):
    layer1: H1T[h, tok] = relu(W1q^T @ XT + b1q)   (PSUM-accum over d-tiles)
    layer2: Y[tok, d]  += H1T^T @ W2q              (PSUM-accum over h-tiles,
                                                    SBUF f32 accum across q)
All matmul operands are bf16 (accumulation stays f32 in PSUM): walrus then
emits separate LDWEIGHTS instructions that the PE pulls into the background
weight buffer, so the ~128-cycle weight load hides under the previous
matmul's streaming instead of serializing with it (f32r matmuls are
self-loading and pay that cost on every MM). W1 is host-retiled to
[ht, p, kt, h] so each h-tile's weights load as one DMA with 2KB
contiguous lines; b1 is host-swizzled so the bias tile loads as 128
contiguous lines instead of 4096 4-byte descriptors.
"""

import math
import numpy as np
import ml_dtypes

import concourse.bass as bass
import concourse.mybir as mybir
import concourse.tile as tile
from concourse import bacc
from concourse.bass_utils import run_bass_kernel_spmd

dt = mybir.dt

B, S, D, H, E, NCORES = 4, 1024, 1024, 4096, 8, 8
HQ = 1024                      # h-quarter width
KT = D // 128                  # 8 contraction tiles (d)
HTQ = HQ // 128                # 8 h-tiles per quarter
NQ = H // HQ                   # 4 quarters
# PE warm-up matmuls: enough continuous PE-busy time (~3.4us) to flip the
# HAM clock gate to 8/8 BEFORE the first data-dependent matmul, and enough
# to bridge the DMA prologue without the PE idling in between — an idle
# gap resets the HAM busy window and leaves the whole first quarter
# running at 1.2 GHz
WARMUP_MMS = 15

BF16 = np.dtype(ml_dtypes.bfloat16)

_BUILD_CACHE: dict = {}


def build_nc(C: int):
    """Build + compile the per-core Bass program for token capacity C."""
    assert C % 128 == 0
    assert NQ >= 2  # final-quarter y store lives in the add branch
    TT = C // 128

    nc = bacc.Bacc(
        "TRN2",
        target_bir_lowering=False,
        debug=False,
        num_devices=NCORES,
    )

    xt_d = nc.dram_tensor("xt", [D, C], dt.bfloat16, kind="ExternalInput")
    # host-retiled: w1[ht, p, kt, h] = W1[kt*128+p, ht*128+h]
    w1_d = nc.dram_tensor(
        "w1", [H // 128, 128, KT, 128], dt.bfloat16, kind="ExternalInput"
    )
    # host-swizzled: b1s[p*32+ht] = b1[ht*128+p]
    b1_d = nc.dram_tensor("b1", [H], dt.float32, kind="ExternalInput")
    w2_d = nc.dram_tensor("w2", [H, D], dt.bfloat16, kind="ExternalInput")
    y_d = nc.dram_tensor("y", [C, D], dt.float32, kind="ExternalOutput")

    xt_v = xt_d.ap().rearrange("(kt p) c -> p kt c", p=128)
    b1_v = b1_d.ap().rearrange("(p ht) -> p ht", p=128)
    y_v = y_d.ap().rearrange("(tt p) d -> p tt d", p=128)
    w2_v = w2_d.ap().rearrange("(hh p) d -> p hh d", p=128)
    w1_4v = w1_d.ap().rearrange("a p k h -> p a k h")

    # SBUF per-partition budget check (bytes)
    need = (
        KT * C * 2                # xt (bf16)
        + TT * 1024 * 4           # y (f32)
        + 2 * HTQ * KT * 128 * 2  # w1 quarters (double-buffered)
        + 2 * HTQ * 1024 * 2      # w2 quarters (double-buffered)
        + 2 * HTQ * C * 2         # h1q (double-buffered)
        + 32 * 4                  # b1
        + 512 * 2                 # warm tile
    )
    assert need <= 200 * 1024, f"SBUF over budget: {need // 1024}KB for C={C}"

    # xt arrives as fused all-kt column-chunk DMAs; the leading chunks are
    # fine so the first L1 h-tile can start computing before the bulk lands
    xt_chunks = [(0, 128), (128, 128), (256, 128), (384, 128)]
    c0 = 512
    while c0 < C:
        n = min(512, C - c0)
        xt_chunks.append((c0, n))
        c0 += n
    n_chunks = [(c0, min(512, C - c0)) for c0 in range(0, C, 512)]

    with tile.TileContext(nc) as tc:
        with (
            tc.tile_pool(name="xt", bufs=1) as xt_pool,
            tc.tile_pool(name="b1", bufs=1) as b1_pool,
            tc.tile_pool(name="y", bufs=1) as y_pool,
            tc.tile_pool(name="w1q", bufs=2) as w1_pool,
            tc.tile_pool(name="w2q", bufs=2) as w2_pool,
            tc.tile_pool(name="h1q", bufs=2) as h1_pool,
            tc.tile_pool(name="ps1", bufs=4, space="PSUM") as ps1_pool,
            tc.tile_pool(name="ps2", bufs=4, space="PSUM") as ps2_pool,
        ):
            # PE warm-up: dependency-free bf16 matmuls issued during the
            # initial DMA prologue so the HAM clock gate reaches 8/8
            # (2.4 GHz) before the first real matmul. The warm tile borrows
            # the y pool's buffer (y is first written long after the warm-up
            # matmuls retire) and the warm PSUM tile rotates through ps2 —
            # no dedicated pools, which keeps the scheduler's end-of-program
            # semaphore quiesce short.
            wt = y_pool.tile([128, 512], dt.bfloat16)
            nc.vector.memset(wt[:], 0.0)
            wps = ps2_pool.tile([128, 512], dt.float32, tag="ps2")
            for _ in range(WARMUP_MMS):
                nc.tensor.matmul(wps[:], wt[:, :128], wt[:], start=True, stop=True)

            # xt chunks are interleaved with the w1 quarter-0 loads below in
            # exact consumption order, all on the sync (HWDGE) ring; the
            # gpsimd SWDGE path stays completely unused. b1 (16KB) rides
            # along early (first activation needs it ~1us after the first
            # real matmul group).
            b1t = b1_pool.tile([128, H // 128], dt.float32)
            xt = xt_pool.tile([128, KT, C], dt.bfloat16)
            y = y_pool.tile([128, TT, 1024], dt.float32)

            # q=0 layer 1 runs chunk-major: sweep columns 0-512 over every
            # h-tile first (fine chunks on ht=0 so compute starts as soon as
            # the first 128 columns land), then the 512+ columns — by then
            # the xt bulk chunks have long finished streaming
            fine_chunks = [(c0, n) for c0, n in xt_chunks if c0 < 512]

            def l1_group(w1q, h1q, q, ht, c0, n):
                hidx = q * HTQ + ht
                ps = ps1_pool.tile([128, 512], dt.float32, tag="ps1")
                for kt in range(KT):
                    nc.tensor.matmul(
                        ps[:, :n],
                        w1q[:, ht, kt, :],
                        xt[:, kt, c0 : c0 + n],
                        start=(kt == 0),
                        stop=(kt == KT - 1),
                    )
                nc.scalar.activation(
                    h1q[:, ht, c0 : c0 + n],
                    ps[:, :n],
                    mybir.ActivationFunctionType.Relu,
                    bias=b1t[:, hidx : hidx + 1],
                )

            for q in range(NQ):
                w2q = w2_pool.tile([128, HTQ, 1024], dt.bfloat16)
                h1q = h1_pool.tile([128, HTQ, C], dt.bfloat16)

                # ---- layer 1: H1T[h, tok] = relu(W1q^T @ XT + b1) ----
                w1q = w1_pool.tile([128, HTQ, KT, 128], dt.bfloat16)
                if q == 0:
                    # prologue rides BOTH HWDGE rings (sync + scalar) in
                    # parallel, each in pass-A consumption order — the
                    # serialized early transfers are the ramp critical path
                    fine = [c for c in xt_chunks if c[0] < 512]
                    bulk = [c for c in xt_chunks if c[0] >= 512]

                    def _xt(c):
                        c0, n = c
                        return (xt[:, :, c0 : c0 + n], xt_v[:, :, c0 : c0 + n])

                    sync_ops = [
                        _xt(fine[0]),
                        (w1q[:, 0], w1_d.ap()[0]),
                        _xt(fine[2]),
                        (w1q[:, 2], w1_d.ap()[2]),
                        (w1q[:, 4], w1_d.ap()[4]),
                        _xt(bulk[0]),
                        (w1q[:, 6], w1_d.ap()[6]),
                    ]
                    scalar_ops = [
                        (b1t[:], b1_v),
                        _xt(fine[1]),
                        (w1q[:, 1], w1_d.ap()[1]),
                        _xt(fine[3]),
                        (w1q[:, 3], w1_d.ap()[3]),
                        (w1q[:, 5], w1_d.ap()[5]),
                        (w1q[:, 7], w1_d.ap()[7]),
                    ] + [_xt(c) for c in bulk[1:]]
                    for i in range(max(len(sync_ops), len(scalar_ops))):
                        if i < len(sync_ops):
                            nc.sync.dma_start(*sync_ops[i])
                        if i < len(scalar_ops):
                            nc.scalar.dma_start(*scalar_ops[i])
                else:
                    # later quarters load 4 h-tiles per DMA — nothing is
                    # latency-critical there, and fewer ops cost less
                    # sequencer time
                    for ht in range(0, HTQ, 4):
                        nc.sync.dma_start(
                            w1q[:, ht : ht + 4], w1_4v[:, q * HTQ + ht : q * HTQ + ht + 4]
                        )
                if q == 0:
                    for ht in range(HTQ):
                        for c0, n in fine_chunks if ht == 0 else n_chunks[:1]:
                            l1_group(w1q, h1q, q, ht, c0, n)
                    for ht in range(HTQ):
                        for c0, n in n_chunks[1:]:
                            l1_group(w1q, h1q, q, ht, c0, n)
                else:
                    for ht in range(HTQ):
                        for c0, n in n_chunks:
                            l1_group(w1q, h1q, q, ht, c0, n)

                # w2 quarter loads (4 h-tiles per DMA), emitted after layer
                # 1 so they never outprioritize the w1 stream on the sync
                # queue; they land well before layer 2 needs them
                for ht in range(0, HTQ, 4):
                    nc.sync.dma_start(
                        w2q[:, ht : ht + 4, :],
                        w2_v[:, q * HTQ + ht : q * HTQ + ht + 4, :],
                    )

                # ---- layer 2: Y[tok, d] += H1T^T @ W2q ----
                for tt in range(TT):
                    for dc in range(2):
                        ps = ps2_pool.tile([128, 512], dt.float32, tag="ps2")
                        for ht in range(HTQ):
                            nc.tensor.matmul(
                                ps[:],
                                h1q[:, ht, tt * 128 : (tt + 1) * 128],
                                w2q[:, ht, dc * 512 : (dc + 1) * 512],
                                start=(ht == 0),
                                stop=(ht == HTQ - 1),
                            )
                        ys = y[:, tt, dc * 512 : (dc + 1) * 512]
                        if q == 0:
                            nc.vector.tensor_copy(ys, ps[:])
                        else:
                            nc.vector.tensor_add(ys, ys, ps[:])
                            if q == NQ - 1:
                                # stores ride the scalar (ACT) HWDGE ring —
                                # a separate physical ring from the sync
                                # one, so they never queue ahead of the
                                # w1/w2 weight stream
                                nc.scalar.dma_start(
                                    y_v[:, tt, dc * 512 : (dc + 1) * 512], ys
                                )

    nc.compile()
    return nc


def _get_nc(C: int):
    if C not in _BUILD_CACHE:
        _BUILD_CACHE[C] = build_nc(C)
    return _BUILD_CACHE[C]


def _retile_w1(w1e: np.ndarray) -> np.ndarray:
    # [D, H] -> [ht, p, kt, h] with w1[ht, p, kt, h] = W1[kt*128+p, ht*128+h]
    return np.ascontiguousarray(
        w1e.reshape(KT, 128, H // 128, 128).transpose(2, 1, 0, 3).astype(BF16)
    )


def kernel(x, W1, b1, W2, b2, assign, k, _want_trace=False):
    x = np.asarray(x, dtype=np.float32)
    W1 = np.asarray(W1, dtype=np.float32)
    b1 = np.asarray(b1, dtype=np.float32)
    W2 = np.asarray(W2, dtype=np.float32)
    b2 = np.asarray(b2, dtype=np.float32)
    assign = np.asarray(assign)
    kk = int(k)

    assert W1.shape[0] == E and W2.shape[0] == E, "expert count must be 8"
    Bx, Sx, Dx = x.shape
    T = Bx * Sx
    xf = x.reshape(T, Dx)
    xT = np.ascontiguousarray(xf.T.astype(BF16))  # [D, T] bf16
    a2 = assign.reshape(T, -1)

    idx = [np.nonzero((a2 == e).any(axis=1))[0] for e in range(E)]
    max_n = max(len(i) for i in idx)

    # capacity per device pass (multiple of 128); single pass for the
    # expected distribution, multiple passes if pathologically skewed
    C = min(max(1024, math.ceil(max_n / 128) * 128), 1280)
    n_pass = math.ceil(max(max_n, 1) / C)

    nc = _get_nc(C)

    w1_io = [_retile_w1(W1[e]) for e in range(E)]
    w2_io = [np.ascontiguousarray(W2[e].astype(BF16)) for e in range(E)]
    # b1s[p*32+ht] = b1[ht*128+p]
    b1_io = [
        np.ascontiguousarray(b1[e].reshape(H // 128, 128).T).reshape(H)
        for e in range(E)
    ]

    out_f = np.zeros((T, Dx), dtype=np.float32)
    trace_info = None

    for p in range(n_pass):
        in_maps = []
        for e in range(E):
            sl = idx[e][p * C : (p + 1) * C]
            xt_buf = np.zeros((Dx, C), dtype=BF16)
            if len(sl):
                xt_buf[:, : len(sl)] = xT[:, sl]
            in_maps.append(
                {
                    "xt": xt_buf,
                    "w1": w1_io[e],
                    "b1": b1_io[e],
                    "w2": w2_io[e],
                }
            )
        res = run_bass_kernel_spmd(
            nc,
            in_maps,
            core_ids=list(range(NCORES)),
            trace=_want_trace,
            trace_cores=list(range(NCORES)) if _want_trace else None,
        )
        if _want_trace:
            trace_info = res
        for e in range(E):
            sl = idx[e][p * C : (p + 1) * C]
            if len(sl):
                out_f[sl] += res.results[e]["y"][: len(sl)] + b2[e][None, :]

    out = (out_f * np.float32(1.0 / kk)).reshape(Bx, Sx, Dx)
    if _want_trace:
        return out, trace_info
    return out
